# revision 2
# baseline (speedup 1.0000x reference)
"""Multi-head attention (nn_Attention_18528488915211) on 8 Trainium2 NeuronCores.

Sharding: tensor-parallel over heads. 16 heads / 8 cores = 2 heads per core.
Each core computes Q/K/V projections for its 256 columns of Wq/Wk/Wv,
attention for its 2 heads, and a partial (transposed) output projection with
its 256 rows of Wo. The host sums the 8 partial outputs (the TP all-reduce),
adds a host-computed rank-1 correction per head, transposes, and adds bo.

Kernel design (v5):
  - Q/K projections in fp8e4m3 with MatmulPerfMode.DoubleRow (weights
    host-scaled by 64; 1/64^2 folded into the softmax exp scale).
  - V projection in fp8 (non-DR; fp8 runs at bf16 speed) -> only the fp8 x
    copy is needed in DRAM (halves x DMA traffic); V output v8 stored in fp8.
  - Attention centering trick: p = 1 + dp with dp = exp(s) - 1 small
    (|dp| ~ 0.07), so fp8 quantization of dp is ~30x more accurate than of
    p.  GpSimd computes dp8 = (pt - 1)*8 in fp8; the AV matmul then runs in
    fp8 DoubleRow over key-block pairs (2x PE throughput).  The missing
    Sum(v) rank-1 term is reconstructed EXACTLY on the host from
    colsum(V) = sum_t(x) @ Wv + L*bv and the exported per-query softmax
    normalizers (1/Sum p).
  - O projection also fp8 DoubleRow: it consumes dev = (Sum dp*v)/(Sum p)
    (the deviation part of prefinal, scaled x128 into fp8 range) against
    64x-scaled Wo pairs over the two heads; scale 2^-13 restored in the
    PSUM->SBUF copy.  The dominant rank-1 part (Wo^T colsum) x (1/Sum p)
    is added on the host (a rank-16 matmul per batch).
  - Softmax denominator from exp(s) in fp16 accumulated on DVE, partition-
    reduced by a 1/16-valued ones-matmul, reciprocal_approx_fast (~18 bits).
  - Phase A of batch b+1 is emitted interleaved into the attention blocks of
    batch b so projection matmuls fill the exp-paced attention gaps.
  - x DRAM layout is chunk-major so every x DMA has 1KB-contiguous
    per-partition runs.
"""

import ml_dtypes
import numpy as np

P = 128          # partitions
DM = 2048        # dmodel
DH = 128         # dhead
HPC = 2          # heads per core
DC = HPC * DH    # dmodel columns per core (256)
B = 4            # batch
L = 2048         # sequence length
T = B * L        # total tokens (8192)
KS8 = DM // 256  # fp8 DoubleRow contraction chunks (8)
TC = 512         # token chunk (projection free dim / PSUM bank)
TI = 1024        # attention query chunk
NCH = L // TC    # chunks per batch (4)
G = T // TC      # global chunks (16)
NJ = L // P      # key blocks per batch (16)
NPAIR = NJ // 2  # key block pairs (8)
NB = DM // P     # output row blocks (16)
NCORES = 8
H = 16           # total heads
SW = 64.0        # fp8 weight scale (q/k/v)
SC = 1.0 / (DH * SW * SW)  # exp scale: 1/dhead (applied twice) and 1/SW^2
S1 = 8.0         # dp8 scale
W1 = 1.0 / 16.0  # ones-matmul value -> dev8 = 128*dev
SW2 = 64.0       # wo8 scale
OSCALE = 1.0 / (SW2 * 128.0)  # restore scale on o-proj copy (2^-13)
AVLAG = 3        # j-lag before the AV matmul consumes a dp8 pair


def _build_nc():
    import concourse.mybir as mybir
    import concourse.tile as tile
    from concourse import bacc

    f32 = mybir.dt.float32
    bf16 = mybir.dt.bfloat16
    fp16 = mybir.dt.float16
    f8 = mybir.dt.float8e4
    EXP = mybir.ActivationFunctionType.Exp
    COPY = mybir.ActivationFunctionType.Copy
    DR = mybir.MatmulPerfMode.DoubleRow
    MULT = mybir.AluOpType.mult
    ADD = mybir.AluOpType.add

    nc = bacc.Bacc("TRN2", target_bir_lowering=False, debug=False,
                   num_devices=NCORES)

    # x8[g, c, p, i, tt] = x^T[(2c+i)*128 + p, g*TC + tt] in fp8
    x8 = nc.dram_tensor("x8", [G, KS8, P, 2, TC], f8, kind="ExternalInput").ap()
    # w[p, c, i, m] = (64*W)[(2c+i)*128 + p, m] in fp8
    wq = nc.dram_tensor("wq", [P, KS8, 2, DC], f8, kind="ExternalInput").ap()
    wk = nc.dram_tensor("wk", [P, KS8, 2, DC], f8, kind="ExternalInput").ap()
    wv = nc.dram_tensor("wv", [DM, DC], f8, kind="ExternalInput").ap()  # *64
    bq = nc.dram_tensor("bq", [DC], f32, kind="ExternalInput").ap()  # *64
    bk = nc.dram_tensor("bk", [DC], f32, kind="ExternalInput").ap()  # *64
    bv = nc.dram_tensor("bv", [DC], f32, kind="ExternalInput").ap()
    # wo8[p, h, n] = (64*Wo)[h*128 + p, n] of this core's 256-row shard
    wo = nc.dram_tensor("wo", [P, HPC, DM], f8, kind="ExternalInput").ap()
    # transposed partial output (dev part): out[n, t]
    out = nc.dram_tensor("out", [DM, T], bf16, kind="ExternalOutput").ap()
    # exported reciprocal rows: orcp[(h*B+b)*4 + ic*2 + u] = 1/(W1*Sum p)
    orcp = nc.dram_tensor("orcp", [HPC * B * 4, TC], f32,
                          kind="ExternalOutput").ap()

    with tile.TileContext(nc) as tc:
        with (
            tc.tile_pool(name="wpool", bufs=1) as wpool,
            tc.tile_pool(name="xpool", bufs=16) as xpool,
            tc.tile_pool(name="qkv", bufs=1) as qkv,
            tc.tile_pool(name="ptp", bufs=4) as ptp,
            tc.tile_pool(name="misc", bufs=2) as misc,
            tc.tile_pool(name="ps", bufs=2, space="PSUM") as ps,
        ):
            # --- resident weights/constants ---
            wq_sb = wpool.tile([P, KS8, 2, DC], f8, tag="wq")
            wk_sb = wpool.tile([P, KS8, 2, DC], f8, tag="wk")
            wv_sb = wpool.tile([P, DM // P, DC], f8, tag="wv")
            wo_sb = wpool.tile([P, HPC, DM], f8, tag="wo")
            bq_sb = wpool.tile([P, HPC], f32, tag="bq")
            bk_sb = wpool.tile([P, HPC], f32, tag="bk")
            bv_sb = wpool.tile([P, 2 * DC], f32, tag="bv")
            ones_sb = wpool.tile([P, P], fp16, tag="ones")
            nc.any.memset(ones_sb[:], W1)

            fetched = {}
            nf = [0]

            def fetch_through(g):
                while nf[0] <= g and nf[0] < G:
                    gg = nf[0]
                    tiles = []
                    for c in range(KS8):
                        x8_t = xpool.tile([P, 2, TC], f8, tag="x8",
                                          name="x8t")
                        nc.sync.dma_start(x8_t[:], x8[gg, c, :, :, :])
                        tiles.append(x8_t)
                    fetched[gg] = tiles
                    nf[0] += 1

            fetch_through(0)
            # weights: c=0 slices first so the first Q/K chunk isn't gated
            for c in range(KS8):
                nc.scalar.dma_start(wq_sb[:, c, :, :], wq[:, c, :, :])
                nc.scalar.dma_start(wk_sb[:, c, :, :], wk[:, c, :, :])
            nc.scalar.dma_start(bq_sb[:], bq.rearrange("(h d) -> d h", d=P))
            nc.scalar.dma_start(bk_sb[:], bk.rearrange("(h d) -> d h", d=P))
            for u in range(2):
                nc.scalar.dma_start(bv_sb[:, u * DC:(u + 1) * DC],
                                    bv[None, :].to_broadcast((P, DC)))
            for ks in range(DM // P):
                nc.scalar.dma_start(wv_sb[:, ks, :],
                                    wv[ks * P:(ks + 1) * P, :])
            fetch_through(1)

            # per-batch SBUF tiles (bufs=2 rotation across batches)
            btiles = {}

            def get_btiles(b):
                if b not in btiles:
                    qt = qkv.tile([P, HPC, L], bf16, tag="qt", name="qt",
                                  bufs=2)
                    kt = qkv.tile([P, HPC, L], bf16, tag="kt", name="kt",
                                  bufs=2)
                    # v8[p, jp, u, h, d] = V[( (2*jp+u)*128 + p ), h*128+d]
                    v8 = qkv.tile([P, NPAIR, 2, HPC, DH], f8, tag="v8",
                                  name="v8", bufs=2)
                    # dev8[p, h, t] = 128 * dev  (prefinal deviation, fp8)
                    dev8 = qkv.tile([P, HPC, L], f8, tag="dev8", name="dev8",
                                    bufs=2)
                    btiles[b] = (qt, kt, v8, dev8)
                return btiles[b]

            def emit_proj_chunk(b, tci):
                g = b * NCH + tci
                fetch_through(g + 1)
                x8s = fetched.pop(g)
                qt_sb, kt_sb, v8_sb, _ = get_btiles(b)
                # Q^T, K^T via fp8 DoubleRow, weights stationary
                for w_sb, o_sb, b_sb in ((wq_sb, qt_sb, bq_sb),
                                         (wk_sb, kt_sb, bk_sb)):
                    for h in range(HPC):
                        acc = ps.tile([P, TC], f32, tag="pa", name="qk",
                                      bufs=2)
                        for c in range(KS8):
                            nc.tensor.matmul(
                                acc[:],
                                w_sb[:, c, :, h * DH:(h + 1) * DH],
                                x8s[c][:],
                                start=(c == 0), stop=(c == KS8 - 1),
                                perf_mode=DR,
                            )
                        nc.vector.tensor_scalar_add(
                            o_sb[:, h, tci * TC:(tci + 1) * TC],
                            acc[:], b_sb[:, h:h + 1],
                        )
                # V in fp8 (non-DR), x stationary; 2 token blocks per PSUM
                for tp in range(TC // P // 2):
                    acc = ps.tile([P, TC], f32, tag="pa", name="vps",
                                  bufs=2)
                    for ti in range(2):
                        tb = 2 * tp + ti
                        for c in range(KS8):
                            for u in range(2):
                                nc.tensor.matmul(
                                    acc[:, ti * DC:(ti + 1) * DC],
                                    x8s[c][:, u, tb * P:(tb + 1) * P],
                                    wv_sb[:, 2 * c + u, :],
                                    start=(c == 0 and u == 0),
                                    stop=(c == KS8 - 1 and u == 1),
                                )
                    # v8 = fp8(acc/64 + bv)
                    nc.vector.scalar_tensor_tensor(
                        v8_sb[:, tci * 2 + tp, :, :, :], acc[:], 1.0 / SW,
                        bv_sb[:], MULT, ADD,
                    )

            # Deferred finalize / O-proj (see baseline): each (h, ic)'s
            # softmax finalize runs a few iterations into the NEXT block's
            # j-loop; each ic's O-projection is emitted after the following
            # block.
            pending_fin = [None]
            pending_op = [None]

            def emit_oproj(arg):
                b_, i0_, dev8_l = arg
                t0_ = b_ * L
                for nb in range(NB):
                    o_ps = ps.tile([P, TI], f32, tag="st", name="o_ps",
                                   bufs=2)
                    for u in range(2):
                        us = slice(u * TC, (u + 1) * TC)
                        nc.tensor.matmul(
                            o_ps[:, us],
                            wo_sb[:, :, nb * P:(nb + 1) * P],
                            dev8_l[:, :, i0_ + u * TC:i0_ + (u + 1) * TC],
                            start=True, stop=True,
                            perf_mode=DR,
                        )
                    oout = misc.tile([P, TI], bf16, tag="oout",
                                     name="oout", bufs=4)
                    if nb % 2 == 0:
                        nc.vector.tensor_scalar_mul(oout[:], o_ps[:], OSCALE)
                    else:
                        nc.scalar.activation(oout[:], o_ps[:], COPY,
                                             scale=OSCALE)
                    nc.sync.dma_start(
                        out[nb * P:(nb + 1) * P, t0_ + i0_:t0_ + i0_ + TI],
                        oout[:])

            def jblock(b, ic, h, qt_sb, kt_sb, v8_sb, dev8_sb):
                i0 = ic * TI
                ot_ps = ps.tile([P, TI], f32, tag="ot", name="ot_ps",
                                bufs=1)
                racc = misc.tile([P, TI], fp16, tag="racc", name="racc",
                                 bufs=2)
                pts = {}
                dps = {}
                for jj in range(NJ + AVLAG + 2):
                    if jj == 2 and pending_fin[0] is not None:
                        pending_fin[0]()
                        pending_fin[0] = None
                    if jj < NJ:
                        j = jj
                        kt_j = kt_sb[:, h, j * P:(j + 1) * P]
                        st2 = ps.tile([P, TI], f32, tag="st", name="st2",
                                      bufs=2)
                        for u in range(2):
                            nc.tensor.matmul(
                                st2[:, u * TC:(u + 1) * TC], kt_j,
                                qt_sb[:, h, i0 + u * TC:i0 + (u + 1) * TC],
                                start=True, stop=True,
                            )
                        pt = ptp.tile([P, TI], fp16, tag="pt", name="pt",
                                      bufs=4)
                        nc.scalar.activation(pt[:], st2[:], EXP, scale=SC)
                        pts[j] = pt
                        if j % 2 == 0:
                            dps[j // 2] = ptp.tile([P, 2, TI], f8, tag="dp",
                                                   name="dp", bufs=3)
                        # dp8 = (pt - 1) * 8 on the (otherwise idle) GpSimd
                        nc.gpsimd.tensor_scalar(
                            dps[j // 2][:, j % 2, :], pt[:], -1.0, S1,
                            ADD, MULT,
                        )
                        # rowsum partials on DVE (denominator Sum p)
                        if j == 1:
                            nc.vector.tensor_add(racc[:], pts[0][:], pt[:])
                        elif j >= 2:
                            nc.vector.tensor_add(racc[:], racc[:], pt[:])
                    if jj >= AVLAG and (jj - AVLAG) % 2 == 1:
                        jp = (jj - AVLAG) // 2
                        if jp < NPAIR:
                            dpt = dps[jp]
                            for u in range(2):
                                us = slice(u * TC, (u + 1) * TC)
                                nc.tensor.matmul(
                                    ot_ps[:, us],
                                    v8_sb[:, jp, :, h, :],
                                    dpt[:, :, us],
                                    start=(jp == 0), stop=(jp == NPAIR - 1),
                                    perf_mode=DR,
                                )

                def fin(racc=racc, ot_ps=ot_ps, h=h, ic=ic, i0=i0,
                        dev8_l=dev8_sb, b=b):
                    rs = ps.tile([P, TI], f32, tag="st", name="rs",
                                 bufs=2)
                    for u in range(2):
                        us = slice(u * TC, (u + 1) * TC)
                        nc.tensor.matmul(rs[:, us], ones_sb[:],
                                         racc[:, us],
                                         start=True, stop=True)
                        rcp = misc.tile([P, TC], f32, tag="rcp",
                                        name="rcp", bufs=2)
                        nc.vector.reciprocal_approx_fast(rcp[:], rs[:, us])
                        nc.vector.tensor_mul(
                            dev8_l[:, h, i0 + u * TC:i0 + (u + 1) * TC],
                            ot_ps[:, us], rcp[:],
                        )
                        ridx = (h * B + b) * 4 + ic * 2 + u
                        nc.sync.dma_start(orcp[ridx:ridx + 1, :],
                                          rcp[0:1, :])
                pending_fin[0] = fin

            # ============ main schedule ============
            # Phase A of batch 0 up front; for b >= 1, phase A chunks are
            # interleaved between batch b-1's attention blocks.
            for tci in range(NCH):
                emit_proj_chunk(0, tci)
            # wo needed only by the first O-projection; load late
            for hh in range(HPC):
                nc.scalar.dma_start(wo_sb[:, hh, :], wo[:, hh, :])

            for b in range(B):
                qt_sb, kt_sb, v8_sb, dev8_sb = get_btiles(b)
                for k, (ic, h) in enumerate(((0, 0), (0, 1), (1, 0), (1, 1))):
                    jblock(b, ic, h, qt_sb, kt_sb, v8_sb, dev8_sb)
                    if pending_op[0] is not None:
                        emit_oproj(pending_op[0])
                        pending_op[0] = None
                    if h == HPC - 1:
                        pending_op[0] = (b, ic * TI, dev8_sb)
                    if b + 1 < B:
                        emit_proj_chunk(b + 1, k)

            # flush the last finalize + output projection
            if pending_fin[0] is not None:
                pending_fin[0]()
                pending_fin[0] = None
            if pending_op[0] is not None:
                emit_oproj(pending_op[0])
                pending_op[0] = None

    nc.compile()
    return nc


_NC_CACHE = None


def kernel(**inputs: np.ndarray) -> np.ndarray:
    from concourse.bass_utils import run_bass_kernel_spmd

    global _NC_CACHE
    x = np.asarray(inputs["x"], dtype=np.float32)
    Wq, bq = np.asarray(inputs["Wq"]), np.asarray(inputs["bq"])
    Wk, bk = np.asarray(inputs["Wk"]), np.asarray(inputs["bk"])
    Wv, bv = np.asarray(inputs["Wv"]), np.asarray(inputs["bv"])
    Wo, bo = np.asarray(inputs["Wo"]), np.asarray(inputs["bo"])

    f8 = ml_dtypes.float8_e4m3

    xt = np.ascontiguousarray(x.reshape(T, DM).T)          # [DM, T]
    # [G, KS8, P, 2, TC] chunk-major
    x8 = np.ascontiguousarray(
        xt.reshape(KS8, 2, P, G, TC).transpose(3, 0, 2, 1, 4).astype(f8))

    def pack_w8(W):  # [DM, DC] -> [P, KS8, 2, DC] fp8, 64-scaled
        Ws = (W * SW).reshape(KS8, 2, P, DC).transpose(2, 0, 1, 3)
        return np.ascontiguousarray(Ws.astype(f8))

    in_maps = []
    for c in range(NCORES):
        sl = slice(c * DC, (c + 1) * DC)
        wo8 = (Wo[sl, :] * SW2).reshape(HPC, P, DM).transpose(1, 0, 2)
        in_maps.append({
            "x8": x8,
            "wq": pack_w8(Wq[:, sl]),
            "wk": pack_w8(Wk[:, sl]),
            "wv": np.ascontiguousarray((Wv[:, sl] * SW).astype(f8)),
            "bq": np.ascontiguousarray(bq[sl] * SW).astype(np.float32),
            "bk": np.ascontiguousarray(bk[sl] * SW).astype(np.float32),
            "bv": np.ascontiguousarray(bv[sl]).astype(np.float32),
            "wo": np.ascontiguousarray(wo8.astype(f8)),
        })

    if _NC_CACHE is None:
        _NC_CACHE = _build_nc()
    res = run_bass_kernel_spmd(_NC_CACHE, in_maps, core_ids=list(range(NCORES)))

    # ---- host reduction: dev partials + rank-1 colsum reconstruction ----
    acc = res.results[0]["out"].astype(np.float32)
    for c in range(1, NCORES):
        acc = acc + res.results[c]["out"].astype(np.float32)

    # exact colsum of V per batch: [B, DM]
    cs_full = x.sum(axis=1).astype(np.float32) @ Wv.astype(np.float32) \
        + L * bv.astype(np.float32)
    # wocs[b, g, n] = sum_d Wo[g*128+d, n] * cs[b, g*128+d]
    wocs = np.einsum(
        "bgd,gdn->bgn",
        cs_full.reshape(B, H, DH),
        Wo.astype(np.float32).reshape(H, DH, DM))

    # invZ[g, b, t] = 1/(Sum_keys p) from exported reciprocal rows
    invZ = np.empty((H, B, L), dtype=np.float32)
    for c in range(NCORES):
        orcp = res.results[c]["orcp"].astype(np.float32)  # [HPC*B*4, TC]
        for h in range(HPC):
            for b in range(B):
                row = orcp[(h * B + b) * 4:(h * B + b) * 4 + 4, :]
                invZ[2 * c + h, b, :] = W1 * row.reshape(L)

    for b in range(B):
        # rank-16 correction: out^T[:, bL:(b+1)L] += wocs[b]^T @ invZ[:, b]
        acc[:, b * L:(b + 1) * L] += wocs[b].T @ invZ[:, b, :]

    acc = acc.T + bo[None, :].astype(np.float32)
    return np.ascontiguousarray(acc).reshape(B, L, DM)


# revision 6
# speedup vs baseline: 1.0040x; 1.0040x over previous
"""Multi-head attention (nn_Attention_18528488915211) on 8 Trainium2 NeuronCores.

Sharding: tensor-parallel over heads. 16 heads / 8 cores = 2 heads per core.
Each core computes Q/K/V projections for its 256 columns of Wq/Wk/Wv,
attention for its 2 heads, and a partial (transposed) output projection with
its 256 rows of Wo. The host sums the 8 partial outputs (the TP all-reduce),
adds a host-computed rank-1 correction per head, transposes, and adds bo.

Kernel design (v6):
  - Q/K projections in fp8e4m3 with MatmulPerfMode.DoubleRow (weights
    host-scaled by 64; 1/64^2 folded into the softmax exp scale).
  - V projection in fp8 (non-DR; fp8 runs at bf16 speed) so only the fp8 x
    copy is needed in DRAM (halves x DMA traffic); V stored as fp8 (v8).
  - Attention centering trick: p = 1 + dp with dp = exp(s) - 1 small
    (|dp| ~ 0.07), so fp8 quantization of dp is ~30x more accurate than of
    p.  dp8 = (pt - 1)*8 is computed in two token-halves (GpSimd + DVE) to
    keep its latency off the critical path; the AV matmul then runs in fp8
    DoubleRow over key-block pairs (2x PE).  The missing Sum(v) rank-1 term
    is reconstructed EXACTLY on the host from colsum(V) = sum_t(x) @ Wv +
    L*bv and the exported per-query softmax normalizers (1/Sum p).
  - O projection also fp8 DoubleRow on dev = (Sum dp*v)/(Sum p) (scaled x128)
    against 64x-scaled Wo head-pairs; scale 2^-13 restored in the PSUM->SBUF
    copy.  The rank-1 part (Wo^T colsum) x (1/Sum p) is added on the host.
  - Softmax denominator from exp(s) in bf16 accumulated on DVE, partition-
    reduced by a 1/16-valued ones-matmul, reciprocal_approx_fast (~18 bits).
  - Q/K bias adds moved to the Scalar engine (Identity activation with
    per-partition bias) - it is idle during projection windows.
  - Fine-grained interleave: projection chunks of batch b+1 and deferred
    O-projection work are split into ~2-4us units on a filler queue, drained
    at fixed slots inside each attention j-loop so the in-order PE stream
    alternates exp-gated attention matmuls with dense filler.
  - x DRAM layout is chunk-major so every x DMA has 1KB-contiguous
    per-partition runs.
"""

import ml_dtypes
import numpy as np

P = 128          # partitions
DM = 2048        # dmodel
DH = 128         # dhead
HPC = 2          # heads per core
DC = HPC * DH    # dmodel columns per core (256)
B = 4            # batch
L = 2048         # sequence length
T = B * L        # total tokens (8192)
KS8 = DM // 256  # fp8 DoubleRow contraction chunks (8)
TC = 512         # token chunk (projection free dim / PSUM bank)
TI = 1024        # attention query chunk
NCH = L // TC    # chunks per batch (4)
G = T // TC      # global chunks (16)
NJ = L // P      # key blocks per batch (16)
NPAIR = NJ // 2  # key block pairs (8)
NB = DM // P     # output row blocks (16)
NCORES = 8
H = 16           # total heads
SW = 64.0        # fp8 weight scale (q/k/v)
SC = 1.0 / (DH * SW * SW)  # exp scale: 1/dhead (applied twice) and 1/SW^2
S1 = 8.0         # dp8 scale
W1 = 1.0 / 16.0  # ones-matmul value -> dev8 = 128*dev
SW2 = 64.0       # wo8 scale
OSCALE = 1.0 / (SW2 * 128.0)  # restore scale on o-proj copy (2^-13)
AVLAG = 4        # j-lag before the AV matmul consumes a dp8 pair


def _build_nc():
    import concourse.mybir as mybir
    import concourse.tile as tile
    from concourse import bacc

    f32 = mybir.dt.float32
    bf16 = mybir.dt.bfloat16
    fp16 = mybir.dt.float16
    f8 = mybir.dt.float8e4
    EXP = mybir.ActivationFunctionType.Exp
    COPY = mybir.ActivationFunctionType.Copy
    IDENT = mybir.ActivationFunctionType.Identity
    DR = mybir.MatmulPerfMode.DoubleRow
    MULT = mybir.AluOpType.mult
    ADD = mybir.AluOpType.add

    nc = bacc.Bacc("TRN2", target_bir_lowering=False, debug=False,
                   num_devices=NCORES)

    # x8[g, c, p, i, tt] = x^T[(2c+i)*128 + p, g*TC + tt] in fp8
    x8 = nc.dram_tensor("x8", [G, KS8, P, 2, TC], f8, kind="ExternalInput").ap()
    # w[p, c, i, m] = (64*W)[(2c+i)*128 + p, m] in fp8
    wq = nc.dram_tensor("wq", [P, KS8, 2, DC], f8, kind="ExternalInput").ap()
    wk = nc.dram_tensor("wk", [P, KS8, 2, DC], f8, kind="ExternalInput").ap()
    wv = nc.dram_tensor("wv", [DM, DC], f8, kind="ExternalInput").ap()  # *64
    bq = nc.dram_tensor("bq", [DC], f32, kind="ExternalInput").ap()  # *64
    bk = nc.dram_tensor("bk", [DC], f32, kind="ExternalInput").ap()  # *64
    bv = nc.dram_tensor("bv", [DC], f32, kind="ExternalInput").ap()
    # wo8[p, h, n] = (64*Wo)[h*128 + p, n] of this core's 256-row shard
    wo = nc.dram_tensor("wo", [P, HPC, DM], f8, kind="ExternalInput").ap()
    # transposed partial output (dev part): out[n, t]
    out = nc.dram_tensor("out", [DM, T], bf16, kind="ExternalOutput").ap()
    # exported reciprocal rows: orcp[(h*B+b)*4 + ic*2 + u] = 1/(W1*Sum p)
    orcp = nc.dram_tensor("orcp", [HPC * B * 4, TC], f32,
                          kind="ExternalOutput").ap()

    with tile.TileContext(nc) as tc:
        with (
            tc.tile_pool(name="wpool", bufs=1) as wpool,
            tc.tile_pool(name="xpool", bufs=24) as xpool,
            tc.tile_pool(name="qkv", bufs=1) as qkv,
            tc.tile_pool(name="ptp", bufs=4) as ptp,
            tc.tile_pool(name="misc", bufs=2) as misc,
            tc.tile_pool(name="ps", bufs=2, space="PSUM") as ps,
        ):
            # --- resident weights/constants ---
            wq_sb = wpool.tile([P, KS8, 2, DC], f8, tag="wq")
            wk_sb = wpool.tile([P, KS8, 2, DC], f8, tag="wk")
            wv_sb = wpool.tile([P, DM // P, DC], f8, tag="wv")
            wo_sb = wpool.tile([P, HPC, DM], f8, tag="wo")
            bq_sb = wpool.tile([P, HPC], f32, tag="bq")
            bk_sb = wpool.tile([P, HPC], f32, tag="bk")
            bv_sb = wpool.tile([P, 2 * DC], f32, tag="bv")
            ones_sb = wpool.tile([P, P], bf16, tag="ones")
            nc.any.memset(ones_sb[:], W1)

            fetched = {}
            nf = [0]

            def fetch_through(g):
                while nf[0] <= g and nf[0] < G:
                    gg = nf[0]
                    tiles = []
                    for c in range(KS8):
                        x8_t = xpool.tile([P, 2, TC], f8, tag="x8",
                                          name="x8t")
                        nc.sync.dma_start(x8_t[:], x8[gg, c, :, :, :])
                        tiles.append(x8_t)
                    fetched[gg] = tiles
                    nf[0] += 1

            fetch_through(0)
            # weights: c=0 slices first so the first Q/K chunk isn't gated
            for c in range(KS8):
                nc.scalar.dma_start(wq_sb[:, c, :, :], wq[:, c, :, :])
                nc.scalar.dma_start(wk_sb[:, c, :, :], wk[:, c, :, :])
            nc.scalar.dma_start(bq_sb[:], bq.rearrange("(h d) -> d h", d=P))
            nc.scalar.dma_start(bk_sb[:], bk.rearrange("(h d) -> d h", d=P))
            for u in range(2):
                nc.scalar.dma_start(bv_sb[:, u * DC:(u + 1) * DC],
                                    bv[None, :].to_broadcast((P, DC)))
            for ks in range(DM // P):
                nc.scalar.dma_start(wv_sb[:, ks, :],
                                    wv[ks * P:(ks + 1) * P, :])
            fetch_through(2)

            # per-batch SBUF tiles (bufs=2 rotation across batches)
            btiles = {}

            def get_btiles(b):
                if b not in btiles:
                    qt = qkv.tile([P, HPC, L], bf16, tag="qt", name="qt",
                                  bufs=2)
                    kt = qkv.tile([P, HPC, L], bf16, tag="kt", name="kt",
                                  bufs=2)
                    # v8[p, jp, u, h, d] = V[((2*jp+u)*128 + p), h*128+d]
                    v8 = qkv.tile([P, NPAIR, 2, HPC, DH], f8, tag="v8",
                                  name="v8", bufs=2)
                    # dev8[p, h, t] = 128 * dev  (prefinal deviation, fp8)
                    dev8 = qkv.tile([P, HPC, L], f8, tag="dev8", name="dev8",
                                    bufs=2)
                    btiles[b] = (qt, kt, v8, dev8)
                return btiles[b]

            # ---- filler units (emitted inside attention j-loops) ----
            filler = []

            def drain_filler(n=None):
                k = len(filler) if n is None else min(n, len(filler))
                for _ in range(k):
                    filler.pop(0)()

            def unit_qk(b, tci, w_sb, o_sb, b_sb, x8s):
                def emit():
                    for h in range(HPC):
                        acc = ps.tile([P, TC], f32, tag="pa", name="qk",
                                      bufs=2)
                        for c in range(KS8):
                            nc.tensor.matmul(
                                acc[:],
                                w_sb[:, c, :, h * DH:(h + 1) * DH],
                                x8s[c][:],
                                start=(c == 0), stop=(c == KS8 - 1),
                                perf_mode=DR,
                            )
                        # bias on the Scalar engine (idle in proj windows)
                        nc.scalar.activation(
                            o_sb[:, h, tci * TC:(tci + 1) * TC],
                            acc[:], IDENT, bias=b_sb[:, h:h + 1],
                        )
                return emit

            def unit_v(b, tci, tp, v8_sb, x8s):
                def emit():
                    acc = ps.tile([P, TC], f32, tag="pa", name="vps",
                                  bufs=2)
                    for ti in range(2):
                        tb = 2 * tp + ti
                        for c in range(KS8):
                            for u in range(2):
                                nc.tensor.matmul(
                                    acc[:, ti * DC:(ti + 1) * DC],
                                    x8s[c][:, u, tb * P:(tb + 1) * P],
                                    wv_sb[:, 2 * c + u, :],
                                    start=(c == 0 and u == 0),
                                    stop=(c == KS8 - 1 and u == 1),
                                )
                    # v8 = fp8(acc/64 + bv)
                    nc.vector.scalar_tensor_tensor(
                        v8_sb[:, tci * 2 + tp, :, :, :], acc[:], 1.0 / SW,
                        bv_sb[:], MULT, ADD,
                    )
                return emit

            def enqueue_proj_chunk(b, tci):
                g = b * NCH + tci
                fetch_through(g + 1)
                x8s = fetched.pop(g)
                qt_sb, kt_sb, v8_sb, _ = get_btiles(b)
                filler.append(unit_v(b, tci, 0, v8_sb, x8s))
                filler.append(unit_v(b, tci, 1, v8_sb, x8s))
                filler.append(unit_qk(b, tci, wq_sb, qt_sb, bq_sb, x8s))
                filler.append(unit_qk(b, tci, wk_sb, kt_sb, bk_sb, x8s))

            def unit_oproj(b_, i0_, dev8_l, nb0):
                t0_ = b_ * L

                def emit():
                    for nb in range(nb0, nb0 + 4):
                        o_ps = ps.tile([P, TI], f32, tag="st", name="o_ps",
                                       bufs=2)
                        for u in range(2):
                            us = slice(u * TC, (u + 1) * TC)
                            nc.tensor.matmul(
                                o_ps[:, us],
                                wo_sb[:, :, nb * P:(nb + 1) * P],
                                dev8_l[:, :, i0_ + u * TC:i0_ + (u + 1) * TC],
                                start=True, stop=True,
                                perf_mode=DR,
                            )
                        oout = misc.tile([P, TI], bf16, tag="oout",
                                         name="oout", bufs=4)
                        if nb % 3 == 2:
                            nc.vector.tensor_scalar_mul(oout[:], o_ps[:],
                                                        OSCALE)
                        else:
                            nc.scalar.activation(oout[:], o_ps[:], COPY,
                                                 scale=OSCALE)
                        nc.sync.dma_start(
                            out[nb * P:(nb + 1) * P,
                                t0_ + i0_:t0_ + i0_ + TI],
                            oout[:])
                return emit

            def enqueue_oproj(b_, i0_, dev8_l):
                for nb0 in range(0, NB, 4):
                    filler.append(unit_oproj(b_, i0_, dev8_l, nb0))

            # Deferred finalize: each (h, ic)'s softmax finalize runs a few
            # iterations into the NEXT block's j-loop.
            pending_fin = [None]

            def jblock(b, ic, h, qt_sb, kt_sb, v8_sb, dev8_sb):
                i0 = ic * TI
                ot_ps = ps.tile([P, TI], f32, tag="ot", name="ot_ps",
                                bufs=1)
                racc = misc.tile([P, TI], bf16, tag="racc", name="racc",
                                 bufs=2)
                pts = {}
                dps = {}
                for jj in range(NJ + AVLAG + 2):
                    if jj == 2 and pending_fin[0] is not None:
                        pending_fin[0]()
                        pending_fin[0] = None
                    if jj < NJ:
                        j = jj
                        kt_j = kt_sb[:, h, j * P:(j + 1) * P]
                        st2 = ps.tile([P, TI], f32, tag="st", name="st2",
                                      bufs=2)
                        for u in range(2):
                            nc.tensor.matmul(
                                st2[:, u * TC:(u + 1) * TC], kt_j,
                                qt_sb[:, h, i0 + u * TC:i0 + (u + 1) * TC],
                                start=True, stop=True,
                            )
                        pt = ptp.tile([P, TI], bf16, tag="pt", name="pt",
                                      bufs=4)
                        nc.scalar.activation(pt[:], st2[:], EXP, scale=SC)
                        pts[j] = pt
                        if j % 2 == 0:
                            dps[j // 2] = ptp.tile([P, 2, TI], f8, tag="dp",
                                                   name="dp", bufs=3)
                        # dp8 = (pt - 1) * 8, split across GpSimd + DVE
                        dpt = dps[j // 2]
                        nc.gpsimd.tensor_scalar(
                            dpt[:, j % 2, 0:TC], pt[:, 0:TC], -1.0, S1,
                            ADD, MULT,
                        )
                        nc.vector.tensor_scalar(
                            dpt[:, j % 2, TC:TI], pt[:, TC:TI], -1.0, S1,
                            ADD, MULT,
                        )
                        # rowsum partials on DVE (denominator Sum p)
                        if j == 1:
                            nc.vector.tensor_add(racc[:], pts[0][:], pt[:])
                        elif j >= 2:
                            nc.vector.tensor_add(racc[:], racc[:], pt[:])
                        if j in (2, 5, 8, 11, 14):
                            drain_filler(1)
                    if jj >= AVLAG and (jj - AVLAG) % 2 == 1:
                        jp = (jj - AVLAG) // 2
                        if jp < NPAIR:
                            dpt = dps[jp]
                            for u in range(2):
                                us = slice(u * TC, (u + 1) * TC)
                                nc.tensor.matmul(
                                    ot_ps[:, us],
                                    v8_sb[:, jp, :, h, :],
                                    dpt[:, :, us],
                                    start=(jp == 0), stop=(jp == NPAIR - 1),
                                    perf_mode=DR,
                                )

                def fin(racc=racc, ot_ps=ot_ps, h=h, ic=ic, i0=i0,
                        dev8_l=dev8_sb, b=b):
                    rs = ps.tile([P, TI], f32, tag="st", name="rs",
                                 bufs=2)
                    for u in range(2):
                        us = slice(u * TC, (u + 1) * TC)
                        nc.tensor.matmul(rs[:, us], ones_sb[:],
                                         racc[:, us],
                                         start=True, stop=True)
                        rcp = misc.tile([P, TC], f32, tag="rcp",
                                        name="rcp", bufs=2)
                        nc.vector.reciprocal_approx_fast(rcp[:], rs[:, us])
                        nc.vector.tensor_mul(
                            dev8_l[:, h, i0 + u * TC:i0 + (u + 1) * TC],
                            ot_ps[:, us], rcp[:],
                        )
                        ridx = (h * B + b) * 4 + ic * 2 + u
                        nc.sync.dma_start(orcp[ridx:ridx + 1, :],
                                          rcp[0:1, :])
                pending_fin[0] = fin

            # ============ main schedule ============
            # Phase A of batch 0 up front; later batches' projection chunks
            # and all O-projections go through the filler queue.
            for tci in range(NCH):
                enqueue_proj_chunk(0, tci)
            drain_filler()
            # wo needed only by the first O-projection; load late
            for hh in range(HPC):
                nc.scalar.dma_start(wo_sb[:, hh, :], wo[:, hh, :])

            for b in range(B):
                qt_sb, kt_sb, v8_sb, dev8_sb = get_btiles(b)
                for k, (ic, h) in enumerate(((0, 0), (0, 1), (1, 0), (1, 1))):
                    if b + 1 < B:
                        enqueue_proj_chunk(b + 1, k)
                    jblock(b, ic, h, qt_sb, kt_sb, v8_sb, dev8_sb)
                    if h == HPC - 1:
                        enqueue_oproj(b, ic * TI, dev8_sb)
                # flush everything except the just-enqueued O-proj units of
                # ic1 (their dev8 input is finalized only at the start of the
                # next batch's first block)
                drain_filler(max(0, len(filler) - 4))

            # flush the last finalize + remaining output projection
            if pending_fin[0] is not None:
                pending_fin[0]()
                pending_fin[0] = None
            drain_filler()

    nc.compile()
    return nc


_NC_CACHE = None


def kernel(**inputs: np.ndarray) -> np.ndarray:
    from concourse.bass_utils import run_bass_kernel_spmd

    global _NC_CACHE
    x = np.asarray(inputs["x"], dtype=np.float32)
    Wq, bq = np.asarray(inputs["Wq"]), np.asarray(inputs["bq"])
    Wk, bk = np.asarray(inputs["Wk"]), np.asarray(inputs["bk"])
    Wv, bv = np.asarray(inputs["Wv"]), np.asarray(inputs["bv"])
    Wo, bo = np.asarray(inputs["Wo"]), np.asarray(inputs["bo"])

    f8 = ml_dtypes.float8_e4m3

    xt = np.ascontiguousarray(x.reshape(T, DM).T)          # [DM, T]
    # [G, KS8, P, 2, TC] chunk-major
    x8 = np.ascontiguousarray(
        xt.reshape(KS8, 2, P, G, TC).transpose(3, 0, 2, 1, 4).astype(f8))

    def pack_w8(W):  # [DM, DC] -> [P, KS8, 2, DC] fp8, 64-scaled
        Ws = (W * SW).reshape(KS8, 2, P, DC).transpose(2, 0, 1, 3)
        return np.ascontiguousarray(Ws.astype(f8))

    in_maps = []
    for c in range(NCORES):
        sl = slice(c * DC, (c + 1) * DC)
        wo8 = (Wo[sl, :] * SW2).reshape(HPC, P, DM).transpose(1, 0, 2)
        in_maps.append({
            "x8": x8,
            "wq": pack_w8(Wq[:, sl]),
            "wk": pack_w8(Wk[:, sl]),
            "wv": np.ascontiguousarray((Wv[:, sl] * SW).astype(f8)),
            "bq": np.ascontiguousarray(bq[sl] * SW).astype(np.float32),
            "bk": np.ascontiguousarray(bk[sl] * SW).astype(np.float32),
            "bv": np.ascontiguousarray(bv[sl]).astype(np.float32),
            "wo": np.ascontiguousarray(wo8.astype(f8)),
        })

    if _NC_CACHE is None:
        _NC_CACHE = _build_nc()
    res = run_bass_kernel_spmd(_NC_CACHE, in_maps, core_ids=list(range(NCORES)))

    # ---- host reduction: dev partials + rank-1 colsum reconstruction ----
    acc = res.results[0]["out"].astype(np.float32)
    for c in range(1, NCORES):
        acc = acc + res.results[c]["out"].astype(np.float32)

    # exact colsum of V per batch: [B, DM]
    cs_full = x.sum(axis=1).astype(np.float32) @ Wv.astype(np.float32) \
        + L * bv.astype(np.float32)
    # wocs[b, g, n] = sum_d Wo[g*128+d, n] * cs[b, g*128+d]
    wocs = np.einsum(
        "bgd,gdn->bgn",
        cs_full.reshape(B, H, DH),
        Wo.astype(np.float32).reshape(H, DH, DM))

    # invZ[g, b, t] = 1/(Sum_keys p) from exported reciprocal rows
    invZ = np.empty((H, B, L), dtype=np.float32)
    for c in range(NCORES):
        orcp = res.results[c]["orcp"].astype(np.float32)  # [HPC*B*4, TC]
        for h in range(HPC):
            for b in range(B):
                row = orcp[(h * B + b) * 4:(h * B + b) * 4 + 4, :]
                invZ[2 * c + h, b, :] = W1 * row.reshape(L)

    for b in range(B):
        # rank-16 correction: out^T[:, bL:(b+1)L] += wocs[b]^T @ invZ[:, b]
        acc[:, b * L:(b + 1) * L] += wocs[b].T @ invZ[:, b, :]

    acc = acc.T + bo[None, :].astype(np.float32)
    return np.ascontiguousarray(acc).reshape(B, L, DM)
